# revision 37
# baseline (speedup 1.0000x reference)
"""Multi-head attention (AttnProcessor) Bass kernel for 8 Trainium2 cores.

Problem: hidden_states [2,2048,1280], Wq/Wk/Wv/Wo [1280,1280], bo [1280],
20 heads x head_dim 64.  out = softmax(q k^T / 8) v @ Wo + bo.

Sharding: 40 (batch, head) units -> 8 cores x 5 heads.  Cores 0-3 take
batch 0, cores 4-7 batch 1; each core gets a 5-head column slice of
Wq/Wk/Wv and the matching row slice of Wo, computes its partial output
projection [S, D], and the host sums the 4 partials per batch and adds bo.

v2 design: single woven stream.  The kernel is ScalarE-bound (softmax
exp = 160 ops x ~1.15us = 184us), so the whole schedule aims at starting
the exp stream as early as possible and never letting it starve:

  - DMA lead-in is trigger-limited (~0.6us per dma_start on the issuing
    engine), so inputs are loaded with ONE batched descriptor per tensor
    per chunk, spread across engines (sync/gpsimd/vector/scalar/tensor
    all issue triggers concurrently at t=0).
  - hs is loaded once into a persistent SBUF tile (no re-DMA for the
    filler qT projections).
  - a minimal startup (qT pair-0 chunk 0, kT pair-0 keys 0:512, v heads
    0,1 keys 0:512) unblocks the first attention unit at ~16us; ALL
    remaining projection work (kT/v for later key-chunks and heads, qT
    for later chunks, output projection of finished chunks) is emitted
    as filler pieces inside the attention stream, ordered by a deadline
    queue so each unit's inputs are always emitted before the unit.
  - heads are processed in row-tiled PAIRS (head 2p in partitions 0:64
    of qT/kT tile p, head 2p+1 in 64:128) so both heads' QK run
    concurrently in different row-groups of the PE array; the 5th head
    duplicates itself into the top half and row-tiles its two kj tiles.
  - scores are computed transposed (S^T = kT-slice x qT, K=hd) so PV
    needs no transpose and the ones-augmented V gives the softmax
    denominator in the same PSUM accumulation chain.
  - the M=64 projection tails (solo head's qT/kT columns) are emitted as
    COLUMN-TILED pairs (tile_position (0,0)/(0,64) auto-derived from the
    PSUM base partition): qT-tail(chunk c) and kT-tail(keys-chunk c) run
    concurrently in the two column halves of the PE array.
  - normalization runs off the PE: DVE copies free the PSUM bank, then
    DVE reciprocal + GpSimd partition_broadcast + DVE multiply; odd
    heads' output is DMA-shifted into the top half of a packed pair tile
    so the output projection contracts K=128 per head pair.
  - y is staged and written as f16 (partials summed in f32 on the host),
    halving output DMA traffic.
"""

import os
import sys

for _p in ("/opt/trn_rl_repo",):
    if _p not in sys.path and os.path.isdir(_p):
        sys.path.append(_p)

import numpy as np

import concourse.bass as bass
from concourse import bacc
import concourse.mybir as mybir
import concourse.tile as tile
from concourse.bass_utils import run_bass_kernel_spmd

F32 = mybir.dt.float32
F16 = mybir.dt.float16

B, S, D = 2, 2048, 1280
HEADS = 20
HD = D // HEADS          # 64
N_CORES = 8
NH = (B * HEADS) // N_CORES  # heads per core = 5
P = 128


def build_nc(s=S, d=D, nh=NH, hd=HD, cw=512):
    """Build the SPMD per-core program."""
    assert d % P == 0 and s % P == 0 and s % cw == 0 and cw % P == 0
    kt = d // P              # contraction tiles for projections
    c = nh * hd              # projection width (320)
    n_cw = s // cw           # q chunks (4)
    n_kj = s // P            # key tiles (16)
    st = s // P              # S tiles of 128
    kpc = cw // P            # key tiles per key-chunk (4)
    n_pairs = nh // 2        # head pairs (2)
    sm_scale = 1.0 / float(np.sqrt(hd))

    nc = bacc.Bacc("TRN2", target_bir_lowering=False)
    hsT = nc.declare_dram_parameter("hsT", [d, s], F16, isOutput=False)
    wq = nc.declare_dram_parameter("wq", [d, c], F16, isOutput=False)
    wk = nc.declare_dram_parameter("wk", [d, c], F16, isOutput=False)
    wv = nc.declare_dram_parameter("wv", [d, c], F16, isOutput=False)
    wo = nc.declare_dram_parameter("wo", [c, d], F16, isOutput=False)
    y = nc.declare_dram_parameter("y", [s, d], F16, isOutput=True)

    hsT_t = hsT[:].rearrange("(ko p) s -> p ko s", p=P)   # [128, kt, s]
    wq_t = wq[:].rearrange("(ko p) c -> p ko c", p=P)
    wk_t = wk[:].rearrange("(ko p) c -> p ko c", p=P)
    wv_t = wv[:].rearrange("(ko p) c -> p ko c", p=P)

    # projection output column chunks: mi 0/1 = head pairs (M=128),
    # mi 2 = solo head tail (M=64)
    mchunks = [(i, min(i + P, c)) for i in range(0, c, P)]
    n_mi = len(mchunks)      # 3

    with tile.TileContext(nc) as tc:
        with tc.tile_pool(name="persist", bufs=1) as persist:
            # ---- persistent SBUF tensors ----
            qT_tiles = [
                persist.tile([P, s], F16, name=f"qT{i}") for i in range(n_mi)
            ]
            kT_tiles = [
                persist.tile([P, s], F16, name=f"kT{i}") for i in range(n_mi)
            ]
            # v with ones column per head: [128, st, nh, hd+1]
            v_aug = persist.tile([P, st, nh, hd + 1], F16, name="v_aug")
            ones_f32 = persist.tile([P, 1], F32, name="ones_f32")
            # hidden states, resident for the whole kernel
            hs_sb = persist.tile([P, kt, s], F16, name="hs_sb")
            wq_sb = persist.tile([P, kt, c], F16, name="wq_sb")
            wk_sb = persist.tile([P, kt, c], F16, name="wk_sb")
            wv_sb = persist.tile([P, kt, c], F16, name="wv_sb")
            wo_pr = persist.tile([P, n_pairs, d], F16, name="wo_pr")
            wo_solo = persist.tile([hd, d], F16, name="wo_solo")

            # ---- DMA lead-in ----
            # DMA issue costs ~4.5ns/packet on the issuing engine AND
            # all packets share the 16 hardware rings, so the critical
            # path (hs chunk-0 k-tiles + the first wk k-tiles, which
            # unblock the startup kT matmuls) is queued first and in
            # small pieces; everything else follows in need-order.
            nc.vector.memset(ones_f32[:], 1.0)
            # dummy exp: pulls the ~2.7us ACT_TABLE_LOAD for the exp
            # table set into the DMA lead-in (before the wq trigger so
            # wq's packets don't race the critical-path transfers)
            warm = persist.tile([1, 1], F32, name="warm")
            nc.scalar.activation(
                warm[:], ones_f32[0:1, 0:1],
                mybir.ActivationFunctionType.Exp,
            )
            # sync: hs chunk 0 (critical), then wq, the later hs
            # chunks, and wo -- all in need-order on ONE queue so their
            # ring packets cannot jump ahead of the critical path.
            # gpsimd: wk pieces (critical) then wv.  scalar: NO
            # triggers (the exp stream must never sit behind one).
            for k0, k1 in ((0, 1), (1, 2), (2, 4), (4, 6), (6, 8), (8, kt)):
                nc.sync.dma_start(
                    out=hs_sb[:, k0:k1, 0:cw], in_=hsT_t[:, k0:k1, 0:cw]
                )
            for k0, k1 in ((0, 3), (3, 6), (6, kt)):
                nc.gpsimd.dma_start(
                    out=wk_sb[:, k0:k1, :], in_=wk_t[:, k0:k1, :]
                )
            for h in range(2):
                hk = kt // 2
                nc.gpsimd.dma_start(
                    out=wv_sb[:, h * hk : (h + 1) * hk, :],
                    in_=wv_t[:, h * hk : (h + 1) * hk, :],
                )
            nc.sync.dma_start(out=wq_sb[:], in_=wq_t[:, :, :])
            for ncw in range(1, n_cw):
                cs0 = ncw * cw
                for h in range(2):
                    hk = kt // 2
                    nc.sync.dma_start(
                        out=hs_sb[:, h * hk : (h + 1) * hk,
                                  cs0 : cs0 + cw],
                        in_=hsT_t[:, h * hk : (h + 1) * hk,
                                  cs0 : cs0 + cw],
                    )
            nc.sync.dma_start(
                out=wo_pr[:],
                in_=wo[0 : 2 * P, :].rearrange("(pr q) d -> q pr d", q=P),
            )
            nc.sync.dma_start(out=wo_solo[:], in_=wo[2 * P : c, :])
            # v_aug ones columns (depend only on ones_f32)
            with nc.allow_low_precision(reason="ones f16"):
                for sg in range(st):
                    nc.vector.tensor_copy(
                        v_aug[:, sg, :, hd : hd + 1],
                        ones_f32[:].to_broadcast((P, nh, 1)),
                    )

            with (
                tc.tile_pool(name="ps_s", bufs=2, space="PSUM") as ps_s_pool,
                tc.tile_pool(name="ps_o", bufs=2, space="PSUM") as ps_o_pool,
                tc.tile_pool(name="ps_fill", bufs=2, space="PSUM") as ps_fill_pool,
                tc.tile_pool(name="exps", bufs=4) as exps_pool,
                tc.tile_pool(name="small", bufs=4) as small_pool,
                tc.tile_pool(name="otile", bufs=2) as otile_pool,
                tc.tile_pool(name="ystage", bufs=4) as ystage_pool,
            ):
                # ============ filler piece generators ============
                # Each piece is a closure emitting ONE PE matmul (plus
                # drains at chain ends).  Pieces of one accumulation
                # chain are contiguous in the queue; the fill pool's 2
                # bufs let one chain drain while the next runs.

                def proj_pieces(w_sb, dst_tiles, mi, ncw):
                    """qT or kT projection chain for (column chunk mi,
                    seq chunk ncw): 10 matmuls N=cw into one fill tile,
                    DVE-drained into the persistent qT/kT tile."""
                    c0, c1 = mchunks[mi]
                    m = c1 - c0
                    cs = slice(ncw * cw, (ncw + 1) * cw)
                    ps_q = ps_fill_pool.tile([P, cw], F32, tag="fill",
                                             name="fill")

                    def mk(k):
                        def go():
                            nc.tensor.matmul(
                                ps_q[0:m, :],
                                w_sb[:, k, c0:c1],
                                hs_sb[:, k, cs],
                                start=(k == 0),
                                stop=(k == kt - 1),
                            )
                            if k == kt - 1:
                                with nc.allow_low_precision(reason="f16 qk"):
                                    nc.vector.tensor_copy(
                                        dst_tiles[mi][0:m, cs], ps_q[0:m, :]
                                    )
                        return go

                    for k in range(kt):
                        yield mk(k)

                def tail_pair_pieces(ncw):
                    """Solo-head (mi=2, M=64) qT and kT projections as a
                    COLUMN-TILED concurrent pair: qT-tail of seq chunk
                    ncw in PE columns 0:64 (PSUM partitions 0:64), kT-
                    tail of keys-chunk ncw in columns 64:128.  After the
                    drain, DMA duplicates each into the top half of the
                    persistent tile for solo-QK row tiling."""
                    c0, c1 = mchunks[2]
                    m = c1 - c0          # 64
                    cs = slice(ncw * cw, (ncw + 1) * cw)
                    ps_t = ps_fill_pool.tile([P, cw], F32, tag="fill",
                                             name="fill")

                    def mk(k):
                        def go():
                            nc.tensor.matmul(
                                ps_t[0:m, :],
                                wq_sb[:, k, c0:c1],
                                hs_sb[:, k, cs],
                                start=(k == 0),
                                stop=(k == kt - 1),
                            )
                            nc.tensor.matmul(
                                ps_t[m:P, :],
                                wk_sb[:, k, c0:c1],
                                hs_sb[:, k, cs],
                                start=(k == 0),
                                stop=(k == kt - 1),
                            )
                            if k == kt - 1:
                                with nc.allow_low_precision(reason="f16 qk"):
                                    nc.vector.tensor_copy(
                                        qT_tiles[2][0:m, cs], ps_t[0:m, :]
                                    )
                                    nc.vector.tensor_copy(
                                        kT_tiles[2][0:m, cs], ps_t[m:P, :]
                                    )
                                with nc.allow_low_precision(reason="f16 qk"):
                                    nc.vector.tensor_copy(
                                        qT_tiles[2][m:P, cs], ps_t[0:m, :]
                                    )
                                    nc.vector.tensor_copy(
                                        kT_tiles[2][m:P, cs], ps_t[m:P, :]
                                    )
                        return go

                    for k in range(kt):
                        yield mk(k)

                def v_pieces(sg):
                    """v projection for seq tile sg, all heads: 10
                    matmuls N=nh*hd into a fill tile, drained into
                    v_aug."""
                    ps_v = ps_fill_pool.tile([P, cw], F32, tag="fill",
                                             name="fill")

                    def mk(k):
                        def go():
                            nc.tensor.matmul(
                                ps_v[:, 0:c],
                                hs_sb[:, k, sg * P : (sg + 1) * P],
                                wv_sb[:, k, :],
                                start=(k == 0),
                                stop=(k == kt - 1),
                            )
                            if k == kt - 1:
                                with nc.allow_low_precision(reason="f16 v"):
                                    nc.vector.tensor_copy(
                                        v_aug[:, sg, :, 0:hd],
                                        ps_v[:, 0:c].rearrange(
                                            "p (h e) -> p h e", h=nh
                                        ),
                                    )
                        return go

                    for k in range(kt):
                        yield mk(k)

                outT_by_chunk = [[] for _ in range(n_cw)]

                def out_proj_pieces(ncw, drain_engine):
                    """Output projection of chunk ncw: per seq-tile, per
                    d-chunk, a 3-matmul chain (two K=128 pair matmuls
                    and one K=64 solo) then a drain + y DMA."""
                    oT_list = outT_by_chunk[ncw]
                    for tt in range(cw // P):
                        t_lo = (ncw * (cw // P) + tt) * P
                        tl = tt * P
                        y_sb = ystage_pool.tile([P, d], F16, tag="y_sb",
                                                name="y_sb")
                        for nn in range(0, d, 512):
                            ne = min(nn + 512, d)
                            ps_y = ps_fill_pool.tile([P, cw], F32,
                                                     tag="fill", name="fill")

                            def mk(j, nn=nn, ne=ne, ps_y=ps_y, y_sb=y_sb,
                                   tl=tl, t_lo=t_lo):
                                def go():
                                    if j < n_pairs:
                                        nc.tensor.matmul(
                                            ps_y[:, 0 : ne - nn],
                                            oT_list[j][:, tl : tl + P],
                                            wo_pr[:, j, nn:ne],
                                            start=(j == 0),
                                            stop=False,
                                        )
                                    else:
                                        nc.tensor.matmul(
                                            ps_y[:, 0 : ne - nn],
                                            oT_list[j][:, tl : tl + P],
                                            wo_solo[:, nn:ne],
                                            start=False,
                                            stop=True,
                                        )
                                        with nc.allow_low_precision(
                                            reason="f16 y"
                                        ):
                                            drain_engine.tensor_copy(
                                                y_sb[:, nn:ne],
                                                ps_y[:, 0 : ne - nn],
                                            )
                                        if ne == d:
                                            nc.sync.dma_start(
                                                out=y[t_lo : t_lo + P, :],
                                                in_=y_sb[:],
                                            )
                                return go

                            for j in range(n_pairs + 1):
                                yield mk(j)

                # ============ attention unit machinery ============
                def emit_qk(u, ps_s, ncw):
                    kind, idx, g = u
                    if kind == "p":
                        for half in range(2):
                            nc.tensor.matmul(
                                ps_s[:, half * cw : (half + 1) * cw],
                                kT_tiles[idx][half * hd : (half + 1) * hd,
                                              g * P : (g + 1) * P],
                                qT_tiles[idx][half * hd : (half + 1) * hd,
                                              ncw * cw : (ncw + 1) * cw],
                                start=True,
                                stop=True,
                            )
                    else:
                        ht = nh // 2
                        for sl in range(2):
                            kj = 2 * g + sl
                            nc.tensor.matmul(
                                ps_s[:, sl * cw : (sl + 1) * cw],
                                kT_tiles[ht][sl * hd : (sl + 1) * hd,
                                             kj * P : (kj + 1) * P],
                                qT_tiles[ht][sl * hd : (sl + 1) * hd,
                                             ncw * cw : (ncw + 1) * cw],
                                start=True,
                                stop=True,
                            )

                def emit_pv(u, ps_oo, expS):
                    kind, idx, g = u
                    if kind == "p":
                        for half in range(2):
                            nc.tensor.matmul(
                                ps_oo[half][:],
                                v_aug[:, g, 2 * idx + half, :],
                                expS[:, half * cw : (half + 1) * cw],
                                start=(g == 0),
                                stop=(g == n_kj - 1),
                            )
                    else:
                        for sl in range(2):
                            kj = 2 * g + sl
                            nc.tensor.matmul(
                                ps_oo[0][:],
                                v_aug[:, kj, nh - 1, :],
                                expS[:, sl * cw : (sl + 1) * cw],
                                start=(kj == 0),
                                stop=(kj == n_kj - 1),
                            )

                def flush_copy(ps_o, last=False):
                    """Free the PSUM bank: DVE copies o_un and den out
                    (den to a partition-0 tile -- custom DVE ops read
                    partition 0 only).  For the LAST flush (no exps
                    left) the copies go to the idle ScalarE instead so
                    they run in parallel with DVE's rcp/mul chain."""
                    o_un = small_pool.tile([hd, cw], F32, tag="o_un",
                                           name="o_un")
                    den = small_pool.tile([1, cw], F32, tag="den", name="den")
                    if last:
                        nc.scalar.activation(
                            o_un[:], ps_o[0:hd, :],
                            mybir.ActivationFunctionType.Copy,
                        )
                        nc.scalar.activation(
                            den[:], ps_o[hd : hd + 1, :],
                            mybir.ActivationFunctionType.Copy,
                        )
                    else:
                        nc.vector.tensor_copy(o_un[:], ps_o[0:hd, :])
                        nc.vector.tensor_copy(den[:], ps_o[hd : hd + 1, :])
                    return o_un, den

                def flush_bcast(den):
                    rcp = small_pool.tile([1, cw], F32, tag="rcp", name="rcp")
                    nc.vector.reciprocal_approx_fast(rcp[:], den[:])
                    rcp_bc = small_pool.tile([hd, cw], F32, tag="rcp_bc",
                                             name="rcp_bc")
                    nc.gpsimd.partition_broadcast(rcp_bc[:], rcp[:])
                    return rcp_bc

                def flush_mul(o_un, rcp_bc, dst, dst_hi):
                    # dst may live on a different partition range than
                    # the sources (odd head -> partitions 64:128): DVE
                    # access patterns carry a start partition per
                    # operand, so no DMA shift is needed
                    with nc.allow_low_precision(reason="f16 attn out"):
                        nc.vector.tensor_mul(dst, o_un[:], rcp_bc[:])

                def flush_done(u, ncw, ps_oo, last=False):
                    kind, idx, g = u
                    if kind == "p" and g == n_kj - 1:
                        oT_pair = otile_pool.tile([P, cw], F16,
                                                  tag=f"oTp{idx}", name="oTp")
                        staged = [flush_copy(ps_oo[0], last),
                                  flush_copy(ps_oo[1], last)]
                        bcs = [flush_bcast(sd[1]) for sd in staged]
                        flush_mul(staged[0][0], bcs[0], oT_pair[0:hd, :],
                                  False)
                        flush_mul(staged[1][0], bcs[1], oT_pair[hd:P, :],
                                  True)
                        outT_by_chunk[ncw].append(oT_pair)
                    elif kind == "s" and g == n_kj // 2 - 1:
                        oT_solo = otile_pool.tile([hd, cw], F16, tag="oTs",
                                                  name="oTs")
                        o_un, den = flush_copy(ps_oo[0], last)
                        flush_mul(o_un, flush_bcast(den), oT_solo[:], False)
                        outT_by_chunk[ncw].append(oT_solo)

                # ============ deadline-ordered filler queue ============
                # Deadlines are GLOBAL unit indices (40 units per chunk:
                # p0 g0-15, p1 g0-15, solo g0-7).  A chain with deadline
                # t is fully emitted before unit t's QK; min_i guards
                # chains whose inputs only exist from a given unit on.
                # Chains are LAZY generators so pool tiles are allocated
                # in emission order, and each chain runs to completion
                # before the next starts (fill pool has 2 bufs).
                fillq = []  # (deadline, min_i, generator)

                def add_chain(deadline, gen, min_i=0):
                    fillq.append((deadline, min_i, gen))

                def ubase(ncw):
                    return 40 * ncw

                # chunk 0 walls ---------------------------------------
                # p0 g in [4c, 4c+4) needs kT mi0 and v for keys-chunk
                # c (v computed full-width per seq tile; deadlines
                # staggered per seq tile to spread the load)
                for kc in range(1, n_cw):
                    add_chain(ubase(0) + 4 * kc - 3,
                              proj_pieces(wk_sb, kT_tiles, 0, kc))
                    for sg in range(kpc):
                        add_chain(ubase(0) + 4 * kc + sg,
                                  v_pieces(kc * kpc + sg))
                # p1 (units 16..31): qT mi1 chunk 0, kT mi1 (all keys,
                # staggered ahead of p1's kj walls)
                add_chain(ubase(0) + 13, proj_pieces(wq_sb, qT_tiles, 1, 0))
                for kc in range(n_cw):
                    add_chain(ubase(0) + 14 + 4 * kc,
                              proj_pieces(wk_sb, kT_tiles, 1, kc))
                # solo (units 32..39): qT/kT tails col-tiled; solo unit
                # g covers kj 2g,2g+1 -> keys-chunk (2g+1)//4
                for ncw in range(n_cw):
                    add_chain(ubase(0) + 30 + 2 * ncw, tail_pair_pieces(ncw))
                # chunks 1..3: qT mi0/mi1 for chunk c (deadline = chunk
                # start), output projection of chunk c-1 spread inside
                # chunk c (min_i: its solo oT flush is emitted during
                # iteration ubase(ncw), so not before ubase(ncw)+1)
                for ncw in range(1, n_cw):
                    # qT for chunk ncw lands during the previous chunk's
                    # solo units so no burst sits on the chunk boundary.
                    # chunk 3 runs solo-FIRST (see chunk_units), so its
                    # solo qT/kT walls move to the chunk start.
                    add_chain(ubase(ncw) - 6,
                              proj_pieces(wq_sb, qT_tiles, 0, ncw))
                    add_chain(ubase(ncw) + 8,
                              proj_pieces(wq_sb, qT_tiles, 1, ncw))
                    # output projection of chunk ncw-1: spread via
                    # budget from ubase+1 on; the late deadline is only
                    # a backstop
                    add_chain(ubase(ncw + 1) - 4,
                              out_proj_pieces(ncw - 1, nc.vector),
                              min_i=ubase(ncw) + 1)
                fillq.sort(key=lambda t: t[0])

                fq_pos = 0
                fq_it = None

                def pop_piece():
                    """Emit one piece from the current chain; advance to
                    the next chain when exhausted.  False if empty."""
                    nonlocal fq_pos, fq_it
                    while True:
                        if fq_pos >= len(fillq):
                            return False
                        if fq_it is None:
                            fq_it = iter(fillq[fq_pos][2])
                        piece = next(fq_it, None)
                        if piece is None:
                            fq_it = None
                            fq_pos += 1
                            continue
                        piece()
                        return True

                def next_deadline():
                    return fillq[fq_pos][0] if fq_pos < len(fillq) else 1 << 30

                def next_min_i():
                    return fillq[fq_pos][1] if fq_pos < len(fillq) else 1 << 30

                # ============ PE pre-warm ============
                # Dummy matmuls on a memset scratch tile keep the PE
                # busy through the DMA lead-in so the HAM clock gate
                # un-throttles (~3.4us of sustained activity) BEFORE
                # the real projections start; their results are never
                # read.
                warm_sb = small_pool.tile([P, cw], F16, tag="warm_sb",
                                          name="warm_sb")
                nc.vector.memset(warm_sb[:], 0.0)
                for _ in range(14):
                    ps_w = ps_fill_pool.tile([P, cw], F32, tag="fill",
                                             name="fill")
                    nc.tensor.matmul(
                        ps_w[:], warm_sb[:, 0:P], warm_sb[:],
                        start=True, stop=True,
                    )

                # ============ startup projections ============
                # kT mi0 keys-chunk 0, v keys 0:512, qT mi0 chunk 0 --
                # the minimum to unblock unit 0, ordered to match DMA
                # arrival (wk first, then wv, then wq on the sync queue
                # behind hs chunk 0).
                for piece in proj_pieces(wk_sb, kT_tiles, 0, 0):
                    piece()
                for sg in range(kpc):
                    for piece in v_pieces(sg):
                        piece()
                for piece in proj_pieces(wq_sb, qT_tiles, 0, 0):
                    piece()

                # ============ the woven stream ============
                def chunk_units(ncw):
                    pair_us = []
                    for pi in range(n_pairs):
                        pair_us += [("p", pi, kj) for kj in range(n_kj)]
                    solo_us = [("s", nh - 1, g) for g in range(n_kj // 2)]
                    if ncw == n_cw - 1:
                        # last chunk runs the solo head FIRST so the
                        # final output projection can be single-pass
                        # (all oT tiles ready when the tail starts)
                        return solo_us + pair_us
                    return pair_us + solo_us

                stream = [
                    (ncw, u) for ncw in range(n_cw) for u in chunk_units(ncw)
                ]
                upc = len(chunk_units(0))  # 40

                # Units are processed in GROUPS of 2: both QKs (and
                # their exps) are emitted back-to-back, then the
                # previous group's PVs.  QK runs in 64x128 row-tiled
                # mode and PV/fills in 128x128 mode -- grouping halves
                # the number of PE tiling-mode transitions (~300ns
                # each, the dominant per-unit overhead).
                GRP = 2
                prev_group = []  # [(u, ncw, ps_oo, expS), ...]
                group = []
                ps_oo = None

                def retire(pg, last=False):
                    for p_u, p_ncw, p_ps_oo, p_expS in pg:
                        emit_pv(p_u, p_ps_oo, p_expS)
                        flush_done(p_u, p_ncw, p_ps_oo, last)

                for i, (ncw, u) in enumerate(stream):
                    kind, idx, g = u
                    # mandatory walls for this unit
                    while next_deadline() <= i:
                        if not pop_piece():
                            break
                    ps_s = ps_s_pool.tile([P, 2 * cw], F32, tag="ps_s",
                                          name="ps_s")
                    emit_qk(u, ps_s, ncw)
                    expS = exps_pool.tile([P, 2 * cw], F16, tag="expS",
                                          name="expS")
                    nc.scalar.activation(
                        expS[:], ps_s[:],
                        mybir.ActivationFunctionType.Exp,
                        scale=sm_scale,
                    )
                    if g == 0:
                        if kind == "p":
                            ps_oo = (
                                ps_o_pool.tile([hd + 1, cw], F32,
                                               tag="ps_o", name="ps_o"),
                                ps_o_pool.tile([hd + 1, cw], F32,
                                               tag="ps_o", name="ps_o"),
                            )
                        else:
                            ps_oo = (
                                ps_o_pool.tile([hd + 1, cw], F32,
                                               tag="ps_o", name="ps_o"),
                            )
                    group.append((u, ncw, ps_oo, expS))
                    if len(group) < GRP and i != len(stream) - 1:
                        continue
                    # previous group's PVs (and flushes), then filler
                    retire(prev_group)
                    prev_group = group
                    group = []
                    in_chunk = i % upc
                    if in_chunk < 3:
                        budget = 2
                    elif i < upc:
                        budget = 12
                    else:
                        budget = 5
                    for _ in range(budget):
                        # respect not-before constraints; otherwise
                        # drain the queue as fast as budget allows
                        if next_min_i() > i:
                            break
                        if not pop_piece():
                            break
                # tail
                retire(prev_group, last=True)
                while pop_piece():
                    pass
                # last chunk's output projection, single-pass: chunk 3
                # ran solo-first, so all three oT tiles are ready by the
                # time the last pair flushes.  Drains alternate between
                # the now-idle ScalarE and DVE; slab-wise y DMAs
                # pipeline with the remaining tail compute.
                fl = outT_by_chunk[n_cw - 1]
                # chunk 3 flush order was [solo, pair0, pair1]; chain
                # each (tt, nn) as pair0 -> solo -> pair1 so only the
                # LAST matmul waits on the final pair's flush and the
                # first two overlap it
                oT_sol, oT_p0, oT_p1 = fl[0], fl[1], fl[2]
                drain_i = 0
                for tt in range(cw // P):
                    t_lo = ((n_cw - 1) * (cw // P) + tt) * P
                    tl = tt * P
                    y_out = ystage_pool.tile([P, d], F16, tag="y_out",
                                             name="y_out")
                    for nn in range(0, d, 512):
                        ne = min(nn + 512, d)
                        ps_y = ps_fill_pool.tile([P, cw], F32,
                                                 tag="fill", name="fill")
                        nc.tensor.matmul(
                            ps_y[:, 0 : ne - nn],
                            oT_p0[:, tl : tl + P],
                            wo_pr[:, 0, nn:ne],
                            start=True,
                            stop=False,
                        )
                        nc.tensor.matmul(
                            ps_y[:, 0 : ne - nn],
                            oT_sol[:, tl : tl + P],
                            wo_solo[:, nn:ne],
                            start=False,
                            stop=False,
                        )
                        nc.tensor.matmul(
                            ps_y[:, 0 : ne - nn],
                            oT_p1[:, tl : tl + P],
                            wo_pr[:, 1, nn:ne],
                            start=False,
                            stop=True,
                        )
                        with nc.allow_low_precision(reason="f16 y"):
                            if drain_i % 2 == 0:
                                nc.scalar.activation(
                                    y_out[:, nn:ne], ps_y[:, 0 : ne - nn],
                                    mybir.ActivationFunctionType.Copy,
                                )
                            else:
                                nc.vector.tensor_copy(
                                    y_out[:, nn:ne], ps_y[:, 0 : ne - nn]
                                )
                        drain_i += 1
                        nc.sync.dma_start(
                            out=y[t_lo : t_lo + P, nn:ne],
                            in_=y_out[:, nn:ne],
                        )
    nc.compile()
    return nc


_NC_CACHE = {}


def _get_nc():
    key = (S, D, NH, HD)
    if key not in _NC_CACHE:
        _NC_CACHE[key] = build_nc()
    return _NC_CACHE[key]


def shard_inputs(hidden_states, Wq, Wk, Wv, Wo):
    """Build the 8 per-core input maps."""
    hs = np.asarray(hidden_states, dtype=np.float32)
    hsT = [np.ascontiguousarray(hs[b].T) for b in range(B)]  # [D, S] each
    Wo = np.asarray(Wo, dtype=np.float32)
    in_maps = []
    cores_per_b = N_CORES // B
    for core in range(N_CORES):
        b = core // cores_per_b
        h0 = (core % cores_per_b) * NH
        cols = slice(h0 * HD, (h0 + NH) * HD)
        in_maps.append(
            {
                "hsT": hsT[b].astype(np.float16),
                "wq": np.ascontiguousarray(np.asarray(Wq, np.float32)[:, cols]).astype(np.float16),
                "wk": np.ascontiguousarray(np.asarray(Wk, np.float32)[:, cols]).astype(np.float16),
                "wv": np.ascontiguousarray(np.asarray(Wv, np.float32)[:, cols]).astype(np.float16),
                "wo": np.ascontiguousarray(Wo[cols, :]).astype(np.float16),
            }
        )
    return in_maps


def kernel(hidden_states, Wq, Wk, Wv, Wo, bo, trace=False):
    nc = _get_nc()
    in_maps = shard_inputs(hidden_states, Wq, Wk, Wv, Wo)
    res = run_bass_kernel_spmd(
        nc, in_maps, core_ids=list(range(N_CORES)), trace=trace
    )
    cores_per_b = N_CORES // B
    out = np.empty((B, S, D), dtype=np.float32)
    bo32 = np.asarray(bo, dtype=np.float32)
    for b in range(B):
        acc = res.results[b * cores_per_b]["y"].astype(np.float32)
        for i in range(1, cores_per_b):
            acc = acc + res.results[b * cores_per_b + i]["y"].astype(np.float32)
        out[b] = acc + bo32
    if trace:
        kernel.last_exec_time_ns = res.exec_time_ns
        kernel.last_results = res
    return out
